# revision 17
# baseline (speedup 1.0000x reference)
"""MoE routing mixture kernel for Trainium2 (8 NeuronCores, SPMD data-parallel).

Math: out[b] = sum_k selection_score[b, idx[b,k]] * all_weight[idx[b,k]]
Rewritten as a dense matmul: out = C @ W_flat, where
  C[b,e]    = selection_score[b,e] * |{k : idx[b,k]==e}|      ([2048, 64])
  W_flat    = all_weight.reshape(64, 16384)
Sharding: batch rows split across 8 cores (256 rows each); W replicated.

Raw Bass (no Tile): this toolchain's descriptors carry at most one sync wait
and one sync update each, so all synchronization is standalone wait_ge
instructions plus .then_inc updates, one per instruction.

Pipeline per core:
  SP   : 6 small input DMAs -> 4 W-chunk DMAs -> 16 output DMAs (1 MiB each)
  DVE  : C = score * count(idx==e) per 128-row chunk; C^T copies from PSUM
  PE   : 2 transposes (C -> C^T), then 64 matmuls [64x128]@[64x512] -> PSUM
  ACT  : 64 PSUM->SBUF copies into 16 staging tiles (no slot reuse)
"""

import sys
from contextlib import ExitStack

import numpy as np

sys.path.insert(0, "/opt/trn_rl_repo")

BS, E, TOPK, PL, D = 2048, 64, 8, 32, 512
NF = PL * D  # 16384 flattened prompt*dim
N_CORES = 8
RPC = BS // N_CORES  # 256 rows per core
RCHUNKS = RPC // 128  # 2 row chunks of 128
HALF = NF // 2  # 8192: W stored on-chip as [128, 8192]
WCHUNKS = 8  # W loaded in 8 chunks of [128, 1024]
WCW = HALF // WCHUNKS  # 2048
SLICES = WCW // D  # 4 matmuls (512 cols) per (chunk, half)
NPSUM = 6  # matmul PSUM ring
NGRP = WCHUNKS * RCHUNKS * 2  # 16 staging groups of [128, 2048]

_cache: dict = {}


def _build_program():
    import concourse.bass as bass
    import concourse.mybir as mybir

    f32 = mybir.dt.float32
    nc = bass.Bass()

    scores_d = nc.declare_dram_parameter("scores", [RPC, E], f32, isOutput=False)
    idx_d = nc.declare_dram_parameter("idxf", [RPC, TOPK], f32, isOutput=False)
    # W_flat [64, 16384] host-rearranged to [128, 8192]:
    # partition h*64+e holds cols [h*8192, (h+1)*8192) of expert e.
    wk_d = nc.declare_dram_parameter("wk", [128, HALF], f32, isOutput=False)
    iota_d = nc.declare_dram_parameter("iota", [128, E], f32, isOutput=False)
    ident_d = nc.declare_dram_parameter("ident", [128, 128], f32, isOutput=False)
    out_d = nc.declare_dram_parameter("out", [RPC, NF], f32, isOutput=True)

    ctx = ExitStack()
    with ctx:
        f32r = mybir.dt.float32r
        sb = lambda shape, tag, dt=f32: ctx.enter_context(  # noqa: E731
            nc.sbuf_tensor(tag, shape, dt)
        )
        w_t = sb([128, HALF], "w_t")
        iota_t = sb([128, E], "iota_t")
        ident_t = sb([128, 128], "ident_t")
        sc_t = [sb([128, E], f"sc{r}") for r in range(RCHUNKS)]
        idx_t = [sb([128, TOPK], f"idx{r}") for r in range(RCHUNKS)]
        eqs = [sb([128, E], f"eq{i}") for i in range(TOPK)]
        prs = [sb([128, E], f"pr{i}") for i in range(TOPK // 2)]
        qds = [sb([128, E], f"qd{i}") for i in range(TOPK // 4)]
        cnt = [sb([128, E], f"cnt{r}") for r in range(RCHUNKS)]
        ct = [sb([128, 128], f"ct{r}") for r in range(RCHUNKS)]
        stg = [sb([128, WCW], f"stg{g}") for g in range(NGRP)]

        ctp = [
            ctx.enter_context(nc.psum_tensor(f"ctp{r}", [E, 128], f32))
            for r in range(RCHUNKS)
        ]
        pmm = [
            ctx.enter_context(nc.psum_tensor(f"pmm{i}", [128, D], f32))
            for i in range(NPSUM)
        ]

        s_in = ctx.enter_context(nc.semaphore("s_in"))
        s_in1 = ctx.enter_context(nc.semaphore("s_in1"))
        s_id = ctx.enter_context(nc.semaphore("s_id"))
        s_w = ctx.enter_context(nc.semaphore("s_w"))
        s_dve = ctx.enter_context(nc.semaphore("s_dve"))
        s_pe = ctx.enter_context(nc.semaphore("s_pe"))
        s_act = ctx.enter_context(nc.semaphore("s_act"))
        s_cpv = ctx.enter_context(nc.semaphore("s_cpv"))
        s_out = ctx.enter_context(nc.semaphore("s_out"))

        # matmul m (PE order) -> (wchunk c, rowchunk rc, half h, slice s)
        def mm_seq():
            m = 0
            for c in range(WCHUNKS):
                for rc in range(RCHUNKS):
                    for h in range(2):
                        for s in range(SLICES):
                            yield m, c, rc, h, s
                            m += 1

        N_MM = WCHUNKS * RCHUNKS * 2 * SLICES  # 64

        block = ctx.enter_context(nc.Block())

        @block.sync
        def _(sp):
            for c in range(WCHUNKS):
                cols = slice(c * WCW, (c + 1) * WCW)
                sp.dma_start(out=w_t[:, cols], in_=wk_d[:, cols]).then_inc(s_w, 16)

        @block.vector
        def _(v):
            for r in range(RCHUNKS):
                if r == 0:
                    v.wait_ge(s_in, 48)
                else:
                    v.wait_ge(s_in1, 32)
                for k in range(TOPK):
                    v.tensor_scalar(
                        eqs[k][:],
                        iota_t[:],
                        idx_t[r][:, k : k + 1],
                        None,
                        mybir.AluOpType.is_equal,
                    )
                v.drain()
                for i in range(TOPK // 2):
                    v.tensor_add(prs[i][:], eqs[2 * i][:], eqs[2 * i + 1][:])
                v.drain()
                for i in range(TOPK // 4):
                    v.tensor_add(qds[i][:], prs[2 * i][:], prs[2 * i + 1][:])
                v.drain()
                v.tensor_add(cnt[r][:], qds[0][:], qds[1][:])
                v.drain()
                v.tensor_mul(cnt[r][:], cnt[r][:], sc_t[r][:]).then_inc(s_dve, 1)
            for r in range(RCHUNKS):
                v.wait_ge(s_pe, r + 1)
                v.tensor_copy(ct[r][:E, :], ctp[r][:]).then_inc(s_dve, 1)
                v.tensor_copy(ct[r][E:, :], ctp[r][:]).then_inc(s_dve, 1)
            # odd-m PSUM->SBUF copies (evens go to ACT)
            for m, c, rc, h, s in mm_seq():
                if m % 2 == 0:
                    continue
                v.wait_ge(s_pe, RCHUNKS + m + 1)
                gi = c * (RCHUNKS * 2) + rc * 2 + h
                v.tensor_copy(
                    stg[gi][:, s * D : (s + 1) * D], pmm[m % NPSUM][:]
                ).then_inc(s_cpv, 1)

        @block.tensor
        def _(t):
            t.wait_ge(s_id, 16)  # ident
            for r in range(RCHUNKS):
                t.wait_ge(s_dve, r + 1)
                t.transpose(ctp[r][:], cnt[r][:], ident_t[:]).then_inc(s_pe, 1)
            t.wait_ge(s_dve, RCHUNKS + 2 * RCHUNKS)  # all ct copies done
            cur_c = -1
            for m, c, rc, h, s in mm_seq():
                if c != cur_c:
                    t.wait_ge(s_w, 16 * (c + 1))
                    cur_c = c
                if m >= NPSUM:
                    mm = m - NPSUM
                    if mm % 2 == 0:
                        t.wait_ge(s_act, mm // 2 + 1)
                    else:
                        t.wait_ge(s_cpv, mm // 2 + 1)
                pslice = slice(h * E, (h + 1) * E)
                wc = c * WCW + s * D
                t.matmul(
                    pmm[m % NPSUM][:],
                    ct[rc][pslice, :],
                    w_t[pslice, wc : wc + D],
                    start=True,
                    stop=True,
                ).then_inc(s_pe, 1)

        @block.scalar
        def _(a):
            for m, c, rc, h, s in mm_seq():
                if m % 2 == 1:
                    continue
                a.wait_ge(s_pe, RCHUNKS + m + 1)
                gi = c * (RCHUNKS * 2) + rc * 2 + h
                a.copy(
                    stg[gi][:, s * D : (s + 1) * D], pmm[m % NPSUM][:]
                ).then_inc(s_act, 1)

        @block.gpsimd
        def _(gp):
            # Small inputs on SWDGE so the HWDGE queue starts W at t=0.
            # Order matters: rc0's tiles first so DVE can start at s_in>=48.
            gp.dma_start(out=iota_t[:], in_=iota_d[:]).then_inc(s_in, 16)
            gp.dma_start(out=sc_t[0][:], in_=scores_d[0:128, :]).then_inc(s_in, 16)
            gp.dma_start(out=idx_t[0][:], in_=idx_d[0:128, :]).then_inc(s_in, 16)
            gp.dma_start(out=ident_t[:], in_=ident_d[:]).then_inc(s_id, 16)
            gp.dma_start(out=sc_t[1][:], in_=scores_d[128:256, :]).then_inc(s_in1, 16)
            gp.dma_start(out=idx_t[1][:], in_=idx_d[128:256, :]).then_inc(s_in1, 16)
            # Output stores on SWDGE: group gi ready when its 2 ACT + 2 DVE
            # copies are done.
            gi = 0
            for c in range(WCHUNKS):
                for rc in range(RCHUNKS):
                    for h in range(2):
                        rows = slice(rc * 128, (rc + 1) * 128)
                        colbase = h * HALF + c * WCW
                        gp.wait_ge(s_act, (SLICES // 2) * (gi + 1))
                        gp.wait_ge(s_cpv, (SLICES // 2) * (gi + 1))
                        gp.dma_start(
                            out=out_d[rows, colbase : colbase + WCW],
                            in_=stg[gi][:],
                        ).then_inc(s_out, 16)
                        gi += 1
            gp.wait_ge(s_out, 16 * NGRP)

    return nc


def _run(selection_score, expert_indices, all_weight, trace=False):
    from concourse.bass_utils import run_bass_kernel_spmd

    scores = np.ascontiguousarray(np.asarray(selection_score, dtype=np.float32))
    idxf = np.ascontiguousarray(np.asarray(expert_indices).astype(np.float32))
    w = np.asarray(all_weight, dtype=np.float32).reshape(E, NF)
    wk = np.ascontiguousarray(
        w.reshape(E, 2, HALF).transpose(1, 0, 2).reshape(128, HALF)
    )
    iota = np.ascontiguousarray(np.tile(np.arange(E, dtype=np.float32), (128, 1)))
    ident = np.eye(128, dtype=np.float32)

    if "nc" not in _cache:
        _cache["nc"] = _build_program()
    nc = _cache["nc"]

    in_maps = [
        {
            "scores": np.ascontiguousarray(scores[c * RPC : (c + 1) * RPC]),
            "idxf": np.ascontiguousarray(idxf[c * RPC : (c + 1) * RPC]),
            "wk": wk,
            "iota": iota,
            "ident": ident,
        }
        for c in range(N_CORES)
    ]
    r = run_bass_kernel_spmd(nc, in_maps, list(range(N_CORES)), trace=trace)
    full = np.concatenate([r.results[c]["out"] for c in range(N_CORES)], axis=0)
    return full.reshape(BS, PL, D).astype(np.float32, copy=False), r


def kernel(selection_score, expert_indices, all_weight) -> np.ndarray:
    full, _ = _run(selection_score, expert_indices, all_weight, trace=False)
    return full


# revision 18
# speedup vs baseline: 1.1718x; 1.1718x over previous
"""MoE routing mixture kernel for Trainium2 (8 NeuronCores, SPMD data-parallel).

Math: out[b] = sum_k selection_score[b, idx[b,k]] * all_weight[idx[b,k]]
Rewritten as a dense matmul: out = C @ W_flat, where
  C[b,e]    = selection_score[b,e] * |{k : idx[b,k]==e}|      ([2048, 64])
  W_flat    = all_weight.reshape(64, 16384)
Sharding: batch rows split across 8 cores (256 rows each); W replicated.

Raw Bass (no Tile): this toolchain's descriptors carry at most one sync wait
and one sync update each, so all synchronization is standalone wait_ge
instructions plus .then_inc updates, one per instruction.

Pipeline per core:
  SP   : 6 small input DMAs -> 4 W-chunk DMAs -> 16 output DMAs (1 MiB each)
  DVE  : C = score * count(idx==e) per 128-row chunk; C^T copies from PSUM
  PE   : 2 transposes (C -> C^T), then 64 matmuls [64x128]@[64x512] -> PSUM
  ACT  : 64 PSUM->SBUF copies into 16 staging tiles (no slot reuse)
"""

import sys
from contextlib import ExitStack

import numpy as np

sys.path.insert(0, "/opt/trn_rl_repo")

BS, E, TOPK, PL, D = 2048, 64, 8, 32, 512
NF = PL * D  # 16384 flattened prompt*dim
N_CORES = 8
RPC = BS // N_CORES  # 256 rows per core
RCHUNKS = RPC // 128  # 2 row chunks of 128
HALF = NF // 2  # 8192: W stored on-chip as [128, 8192]
WCHUNKS = 8  # W loaded in 8 chunks of [128, 1024]
WCW = HALF // WCHUNKS  # 2048
SLICES = WCW // D  # 4 matmuls (512 cols) per (chunk, half)
NPSUM = 6  # matmul PSUM ring
NGRP = WCHUNKS * RCHUNKS * 2  # 16 staging groups of [128, 2048]

_cache: dict = {}


def _build_program():
    import concourse.bass as bass
    import concourse.mybir as mybir

    f32 = mybir.dt.float32
    nc = bass.Bass()

    scores_d = nc.declare_dram_parameter("scores", [RPC, E], f32, isOutput=False)
    idx_d = nc.declare_dram_parameter("idxf", [RPC, TOPK], f32, isOutput=False)
    # W_flat [64, 16384] host-rearranged to [128, 8192]:
    # partition h*64+e holds cols [h*8192, (h+1)*8192) of expert e.
    wk_d = nc.declare_dram_parameter("wk", [128, HALF], f32, isOutput=False)
    iota_d = nc.declare_dram_parameter("iota", [128, E], f32, isOutput=False)
    ident_d = nc.declare_dram_parameter("ident", [128, 128], f32, isOutput=False)
    out_d = nc.declare_dram_parameter("out", [RPC, NF], f32, isOutput=True)

    ctx = ExitStack()
    with ctx:
        f32r = mybir.dt.float32r
        sb = lambda shape, tag, dt=f32: ctx.enter_context(  # noqa: E731
            nc.sbuf_tensor(tag, shape, dt)
        )
        w_t = sb([128, HALF], "w_t")
        iota_t = sb([128, E], "iota_t")
        ident_t = sb([128, 128], "ident_t")
        sc_t = [sb([128, E], f"sc{r}") for r in range(RCHUNKS)]
        idx_t = [sb([128, TOPK], f"idx{r}") for r in range(RCHUNKS)]
        eqs = [sb([128, E], f"eq{i}") for i in range(TOPK)]
        prs = [sb([128, E], f"pr{i}") for i in range(TOPK // 2)]
        qds = [sb([128, E], f"qd{i}") for i in range(TOPK // 4)]
        cnt = [sb([128, E], f"cnt{r}") for r in range(RCHUNKS)]
        ct = [sb([128, 128], f"ct{r}") for r in range(RCHUNKS)]
        stg = [sb([128, WCW], f"stg{g}") for g in range(NGRP)]

        ctp = [
            ctx.enter_context(nc.psum_tensor(f"ctp{r}", [E, 128], f32))
            for r in range(RCHUNKS)
        ]
        pmm = [
            ctx.enter_context(nc.psum_tensor(f"pmm{i}", [128, D], f32))
            for i in range(NPSUM)
        ]

        s_in = ctx.enter_context(nc.semaphore("s_in"))
        s_w = ctx.enter_context(nc.semaphore("s_w"))
        s_dve = ctx.enter_context(nc.semaphore("s_dve"))
        s_pe = ctx.enter_context(nc.semaphore("s_pe"))
        s_act = ctx.enter_context(nc.semaphore("s_act"))
        s_cpv = ctx.enter_context(nc.semaphore("s_cpv"))
        s_out = ctx.enter_context(nc.semaphore("s_out"))

        # matmul m (PE order) -> (wchunk c, rowchunk rc, half h, slice s)
        def mm_seq():
            m = 0
            for c in range(WCHUNKS):
                for rc in range(RCHUNKS):
                    for h in range(2):
                        for s in range(SLICES):
                            yield m, c, rc, h, s
                            m += 1

        N_MM = WCHUNKS * RCHUNKS * 2 * SLICES  # 64

        block = ctx.enter_context(nc.Block())

        @block.sync
        def _(sp):
            sp.dma_start(out=iota_t[:], in_=iota_d[:]).then_inc(s_in, 16)
            sp.dma_start(out=ident_t[:], in_=ident_d[:]).then_inc(s_in, 16)
            for r in range(RCHUNKS):
                rows = slice(r * 128, (r + 1) * 128)
                sp.dma_start(out=sc_t[r][:], in_=scores_d[rows, :]).then_inc(s_in, 16)
                sp.dma_start(out=idx_t[r][:], in_=idx_d[rows, :]).then_inc(s_in, 16)
            for c in range(WCHUNKS):
                cols = slice(c * WCW, (c + 1) * WCW)
                sp.dma_start(out=w_t[:, cols], in_=wk_d[:, cols]).then_inc(s_w, 16)

        @block.vector
        def _(v):
            v.wait_ge(s_in, 96)
            for r in range(RCHUNKS):
                for k in range(TOPK):
                    v.tensor_scalar(
                        eqs[k][:],
                        iota_t[:],
                        idx_t[r][:, k : k + 1],
                        None,
                        mybir.AluOpType.is_equal,
                    )
                v.drain()
                for i in range(TOPK // 2):
                    v.tensor_add(prs[i][:], eqs[2 * i][:], eqs[2 * i + 1][:])
                v.drain()
                for i in range(TOPK // 4):
                    v.tensor_add(qds[i][:], prs[2 * i][:], prs[2 * i + 1][:])
                v.drain()
                v.tensor_add(cnt[r][:], qds[0][:], qds[1][:])
                v.drain()
                v.tensor_mul(cnt[r][:], cnt[r][:], sc_t[r][:]).then_inc(s_dve, 1)
            for r in range(RCHUNKS):
                v.wait_ge(s_pe, r + 1)
                v.tensor_copy(ct[r][:E, :], ctp[r][:]).then_inc(s_dve, 1)
                v.tensor_copy(ct[r][E:, :], ctp[r][:]).then_inc(s_dve, 1)
            # odd-m PSUM->SBUF copies (evens go to ACT)
            for m, c, rc, h, s in mm_seq():
                if m % 2 == 0:
                    continue
                v.wait_ge(s_pe, RCHUNKS + m + 1)
                gi = c * (RCHUNKS * 2) + rc * 2 + h
                v.tensor_copy(
                    stg[gi][:, s * D : (s + 1) * D], pmm[m % NPSUM][:]
                ).then_inc(s_cpv, 1)

        @block.tensor
        def _(t):
            t.wait_ge(s_in, 96)  # ident
            for r in range(RCHUNKS):
                t.wait_ge(s_dve, r + 1)
                t.transpose(ctp[r][:], cnt[r][:], ident_t[:]).then_inc(s_pe, 1)
            t.wait_ge(s_dve, RCHUNKS + 2 * RCHUNKS)  # all ct copies done
            cur_c = -1
            for m, c, rc, h, s in mm_seq():
                if c != cur_c:
                    t.wait_ge(s_w, 16 * (c + 1))
                    cur_c = c
                if m >= NPSUM:
                    mm = m - NPSUM
                    if mm % 2 == 0:
                        t.wait_ge(s_act, mm // 2 + 1)
                    else:
                        t.wait_ge(s_cpv, mm // 2 + 1)
                pslice = slice(h * E, (h + 1) * E)
                wc = c * WCW + s * D
                t.matmul(
                    pmm[m % NPSUM][:],
                    ct[rc][pslice, :],
                    w_t[pslice, wc : wc + D],
                    start=True,
                    stop=True,
                ).then_inc(s_pe, 1)

        @block.scalar
        def _(a):
            for m, c, rc, h, s in mm_seq():
                if m % 2 == 1:
                    continue
                a.wait_ge(s_pe, RCHUNKS + m + 1)
                gi = c * (RCHUNKS * 2) + rc * 2 + h
                a.copy(
                    stg[gi][:, s * D : (s + 1) * D], pmm[m % NPSUM][:]
                ).then_inc(s_act, 1)

        @block.gpsimd
        def _(gp):
            # Output stores on SWDGE: group gi ready when its 2 ACT + 2 DVE
            # copies are done.
            gi = 0
            for c in range(WCHUNKS):
                for rc in range(RCHUNKS):
                    for h in range(2):
                        rows = slice(rc * 128, (rc + 1) * 128)
                        colbase = h * HALF + c * WCW
                        gp.wait_ge(s_act, (SLICES // 2) * (gi + 1))
                        gp.wait_ge(s_cpv, (SLICES // 2) * (gi + 1))
                        gp.dma_start(
                            out=out_d[rows, colbase : colbase + WCW],
                            in_=stg[gi][:],
                        ).then_inc(s_out, 16)
                        gi += 1
            gp.wait_ge(s_out, 16 * NGRP)

    return nc


def _run(selection_score, expert_indices, all_weight, trace=False):
    from concourse.bass_utils import run_bass_kernel_spmd

    scores = np.ascontiguousarray(np.asarray(selection_score, dtype=np.float32))
    idxf = np.ascontiguousarray(np.asarray(expert_indices).astype(np.float32))
    w = np.asarray(all_weight, dtype=np.float32).reshape(E, NF)
    wk = np.ascontiguousarray(
        w.reshape(E, 2, HALF).transpose(1, 0, 2).reshape(128, HALF)
    )
    iota = np.ascontiguousarray(np.tile(np.arange(E, dtype=np.float32), (128, 1)))
    ident = np.eye(128, dtype=np.float32)

    if "nc" not in _cache:
        _cache["nc"] = _build_program()
    nc = _cache["nc"]

    in_maps = [
        {
            "scores": np.ascontiguousarray(scores[c * RPC : (c + 1) * RPC]),
            "idxf": np.ascontiguousarray(idxf[c * RPC : (c + 1) * RPC]),
            "wk": wk,
            "iota": iota,
            "ident": ident,
        }
        for c in range(N_CORES)
    ]
    r = run_bass_kernel_spmd(nc, in_maps, list(range(N_CORES)), trace=trace)
    full = np.concatenate([r.results[c]["out"] for c in range(N_CORES)], axis=0)
    return full.reshape(BS, PL, D).astype(np.float32, copy=False), r


def kernel(selection_score, expert_indices, all_weight) -> np.ndarray:
    full, _ = _run(selection_score, expert_indices, all_weight, trace=False)
    return full
